# revision 11
# baseline (speedup 1.0000x reference)
"""Laplacian normalization kernel for Trainium2 (8 NeuronCores, SPMD).

out = D^-1/2 A D^-1/2 where D = diag(row sums of A), A: [8192, 8192] fp32.

Gate is max elementwise rel-err < 2e-2; bf16 rounding (~1.5% worst case
through the whole chain) sits under it, so the kernel runs bf16 end to
end: A is downcast on the host, each core's 16MB block lives fully
resident in SBUF, and the output is stored bf16 and widened on the host.
HBM traffic per core: 16MB in + 16MB out (vs 80MB for the fp32 kernel).

Sharding: core k owns global rows [512k, 512k+512) and [4096+512k,
4096+512k+512), permuted on the host so that local row p*4 + s_rel + 512g
sits in stripe 4g+s_rel at partition p. Two payoffs:
  - AG half g's output is isq for the contiguous global rows
    [4096g, 4096g+4096) = the column scales for one contiguous half of
    the matrix (dense step-1 compute and stores everywhere), and
  - the per-half isq vector leaves SBUF partition-major, so the DRAM
    write before the AllGather is 128 contiguous 8B descriptors instead
    of a 512-descriptor 2B scatter (which cost ~10us of SDMA drain).

Stripes are packed two-per-SBUF-tile so the 16MB block loads as 4x 4MB
DMAs (two per HWDGE ring, concurrent): stripes 0-3 are resident by
~22us and all 8 by ~43us, so both AllGather triggers beat the ~60-75us
NRT startup rendezvous that floors when collectives can run.

Measured DVE perf modes drove the op selection: tensor_tensor and plain
tensor_scalar hit 2x/4x on bf16, but anything with an accumulator or a
scalar pointer (scalar_tensor_tensor, tensor_scalar+accum, activation)
runs 1x. So:
  row sums: accum side-outputs of in-place identity ops, split DVE/ACT.
  isq: DVE reciprocal -> ACT sqrt, written twice: once straight to the
    bf16 AG payload (no DVE round-trip: a cross-engine copy here made
    the Tile scheduler chain AG1's trigger behind later DVE sums, +24us)
    and once fp32 for the row pre-scales.
  row scale: plain 4x tensor_scalar in place during the AG window.
  column scale: plain 2x tensor_mul against the HWDGE-broadcast AG
    output (cb0 sync ring, cb1 scalar ring; SWDGE broadcast costs 14us).
Stores alternate rings per (stripe, half) unit.
"""

import sys

sys.path.insert(0, "/opt/trn_rl_repo")

import numpy as np

import concourse.bacc as bacc
import concourse.tile as tile
from concourse import mybir
from concourse.bass_utils import run_bass_kernel_spmd

N = 8192          # full matrix dim
CORES = 8
R = N // CORES    # rows per core: 1024
P = 128           # partitions
S = R // P        # row stripes per core: 8
HC = N // 2       # columns covered per AG half: 4096
HAG = R // 2      # isq elements per collective half: 512
HS = S // 2       # stripes per half: 4
F32 = mybir.dt.float32
BF16 = mybir.dt.bfloat16
MUL = mybir.AluOpType.mult
ADD = mybir.AluOpType.add

_CACHE = {}


def build_nc():
    if "nc" in _CACHE:
        return _CACHE["nc"]
    nc = bacc.Bacc(
        "TRN2", target_bir_lowering=False, debug=False, num_devices=CORES
    )
    a = nc.dram_tensor("a_block", [R, N], BF16, kind="ExternalInput").ap()
    out = nc.dram_tensor("out_block", [R, N], BF16, kind="ExternalOutput").ap()

    with tile.TileContext(nc) as tc:
        with (
            tc.tile_pool(name="dram", bufs=1, space="DRAM") as dram,
            tc.tile_pool(name="res", bufs=1) as res,
            tc.tile_pool(name="cpool", bufs=1) as cpool,
            tc.tile_pool(name="small", bufs=1) as small,
        ):
            isq_loc = [
                dram.tile([HAG], BF16, name=f"isq_loc{g}") for g in range(2)
            ]
            isq_ag = [
                dram.tile(
                    [CORES * HAG], BF16, addr_space="Shared", name=f"isq_ag{g}"
                )
                for g in range(2)
            ]

            part = small.tile([P, S], F32)      # row sums (degree)
            inv = small.tile([P, S], F32)       # 1/degree
            isq_sb = small.tile([P, S], F32)    # 1/sqrt(degree), fp32
            isqp = [
                small.tile([P, HS], BF16, name=f"isqp{g}") for g in range(2)
            ]
            warm = small.tile([P, 1], F32)      # sqrt table warmup

            # two stripes per tile: stripe s = a2[s//2][:, (s%2)*N:...]
            a2 = [
                res.tile([P, 2 * N], BF16, tag=f"res{t}", bufs=1, name=f"a2_{t}")
                for t in range(S // 2)
            ]
            cb = [
                cpool.tile([P, HC], BF16, tag=f"cb{g}", bufs=1, name=f"cb{g}")
                for g in range(2)
            ]

            def astripe(s):
                return a2[s // 2][:, (s % 2) * N : (s % 2 + 1) * N]

            # hoist the Sqrt ACT table load off the isq critical path
            nc.scalar.sqrt(warm[:], warm[:])

            # 4MB double-stripe loads, two per ring, dispatched up front;
            # stripes 0-3 (which gate AG1) land first, all by ~43us
            for t in range(S // 2):
                ld = nc.sync if t % 2 == 0 else nc.scalar
                ld.dma_start(
                    a2[t][:].rearrange("p (two c) -> p two c", two=2),
                    a[2 * t * P : (2 * t + 2) * P, :].rearrange(
                        "(two p) c -> p two c", two=2
                    ),
                )

            def row_sum(s):
                """Row sum as accum side-output of an in-place identity op,
                split DVE/ACT (both run 1x; fast enough to beat the NRT
                startup rendezvous, which is all pass 1 has to do)."""
                if s % 2 == 0:
                    nc.vector.tensor_scalar(
                        out=astripe(s),
                        in0=astripe(s),
                        scalar1=1.0,
                        scalar2=None,
                        op0=MUL,
                        op1=ADD,
                        accum_out=part[:, s : s + 1],
                    )
                else:
                    nc.scalar.activation(
                        out=astripe(s),
                        in_=astripe(s),
                        func=mybir.ActivationFunctionType.Copy,
                        accum_out=part[:, s : s + 1],
                    )

            def finish_half(g):
                """part[:, 4g:4g+4] -> isq -> DRAM -> AllGather -> cb[g]."""
                s0 = HS * g
                nc.vector.reciprocal(
                    inv[:, s0 : s0 + HS], part[:, s0 : s0 + HS]
                )
                # bf16 AG payload written directly by ACT, fp32 copy for
                # the row pre-scales
                nc.scalar.sqrt(isqp[g][:], inv[:, s0 : s0 + HS])
                nc.scalar.sqrt(
                    isq_sb[:, s0 : s0 + HS], inv[:, s0 : s0 + HS]
                )
                # isq_loc[g][p*4 + s] = isq of local row 512g + p*4 + s:
                # partition-major, 128 contiguous 8B descriptors (SWDGE -
                # the HWDGE rings still have 4MB loads queued)
                nc.gpsimd.dma_start(
                    isq_loc[g].rearrange("(p s) -> p s", s=HS), isqp[g][:]
                )
                nc.gpsimd.collective_compute(
                    "AllGather",
                    mybir.AluOpType.bypass,
                    ins=[isq_loc[g][:].opt()],
                    outs=[isq_ag[g][:].opt()],
                    replica_groups=[list(range(CORES))],
                )
                # column scales for global columns [4096g, 4096g+4096):
                # partition-broadcast of the AG output on an HWDGE ring
                ring = nc.sync if g == 0 else nc.scalar
                ring.dma_start(
                    cb[g][:], isq_ag[g][:].unsqueeze(0).to_broadcast([P, HC])
                )

            def row_prescale(s):
                """astripe(s) *= isq_row, in place (4x tensor_scalar),
                while the AGs are in flight."""
                nc.vector.tensor_scalar(
                    out=astripe(s),
                    in0=astripe(s),
                    scalar1=isq_sb[:, s : s + 1],
                    scalar2=None,
                    op0=MUL,
                )

            for s in range(HS):
                row_sum(s)
            finish_half(0)
            for s in range(HS):
                row_prescale(s)
            for s in range(HS, S):
                row_sum(s)
            finish_half(1)
            for s in range(HS, S):
                row_prescale(s)

            # pass 2: plain bf16 tensor_mul against the broadcast column
            # scales (2x DVE mode), one [128, 4096] op per (stripe, half);
            # stores alternate rings
            for g in range(2):
                for s in range(S):
                    sl = slice(g * HC, (g + 1) * HC)
                    src = astripe(s)[:, sl]
                    nc.vector.tensor_mul(src, src, cb[g][:])
                    st = nc.sync if (s + g) % 2 == 0 else nc.scalar
                    st.dma_start(out[s * P : (s + 1) * P, sl], src)

    nc.compile()
    _CACHE["nc"] = nc
    return nc


def _perm():
    """gidx[d] = global row held at device row d of core k (add k*512).

    Device row d = 128*s + p; half g = s//4, s_rel = s%4; local row
    u = 512g + 4p + s_rel; global row = k*512 + u for u < 512 else
    4096 + k*512 + (u - 512)."""
    d = np.arange(R)
    s, p = d // P, d % P
    g, s_rel = s // HS, s % HS
    u = 512 * g + 4 * p + s_rel
    return np.where(u < HAG, u, HC + (u - HAG))


_GIDX = _perm()


def make_in_maps(A):
    """Permuted row shard, downcast to bf16 on the host."""
    import ml_dtypes

    return [
        {"a_block": A[_GIDX + k * HAG].astype(ml_dtypes.bfloat16)}
        for k in range(CORES)
    ]


def kernel(adjacency_matrix):
    A = np.ascontiguousarray(np.asarray(adjacency_matrix, dtype=np.float32))
    assert A.shape == (N, N)
    nc = build_nc()
    res = run_bass_kernel_spmd(nc, make_in_maps(A), list(range(CORES)))
    out = np.empty((N, N), dtype=np.float32)
    for k in range(CORES):
        blk = np.asarray(res.results[k]["out_block"]).astype(np.float32)
        out[_GIDX + k * HAG] = blk
    return out
